# revision 36
# baseline (speedup 1.0000x reference)
"""Trainium2 Bass kernel for a 2-layer GAT (PyG GATConv semantics).

Strategy (8 NeuronCores, SPMD, dst-sharded graph parallel, identity-rounds):
  - Host relabels nodes: global in-degree sort (incl. self-loop), snake-deal
    to 8 cores, contiguous 128-node blocks per core (98 blocks). Block b's
    round count R_b = max in-degree in block b across cores (identical
    program on all cores). Edge slot (block b, round k, dst slot j) holds
    the k-th in-edge of dst j; pad slots get hd=0 / z=-300.
  - With this layout the scatter matrix is the CONSTANT IDENTITY: the PE
    accumulates agg[j,:] += hs[round k][j,:] over rounds via PSUM
    accumulation (no per-edge one-hot stream, no per-tile stationary).
  - Launch A (dense): feat = W1ext.T @ x per core shard in bf16; W1ext
    packs W1 (columns permuted to f=4c+h interleaved head order) plus
    per-head attention columns. Outputs hd as bf16 and als/ald as f32.
  - Host gathers per-edge streams into [128 slot, TOT_R, F] grids:
    hd[src] bf16 and z = als[src]+ald[dst] bf16 (pad z = -300 -> ex ~ 0).
  - Launch B (L1 edge phase): per group of ~4 blocks: leaky = max(z, .2z),
    exp on ACT into hs ex columns, hs = hd * ex via one broadcast
    tensor_tensor (4c+h order keeps innermost stride 1 -> DVE 2x mode),
    identity matmuls accumulate agg|den in PSUM per block, epilogue:
    ACT reciprocal, fused relu+normalize, transpose, W2ext -> h2|als2|ald2.
  - Host gathers L2 per-edge streams; Launch C = L2 edge phase -> out2.
All FLOPs happen on device; the host only permutes/gathers/casts (and
adds the two gathered attention-logit streams).
"""

import os
import numpy as np
import ml_dtypes

N_NODES = 100000
N_EDGES = 1600000
IN_DIM = 128
HID = 128
HEADS = 4
C1 = 32
OUT_DIM = 64
NEG = 0.2
NC = 8
NODES_PER_CORE = 12544  # 98 blocks * 128
N_BLOCKS = 98
REAL_PER_CORE = 12500
N_PAD = NC * NODES_PER_CORE
NGROUPS = 33
ZPAD = -300.0

BF16 = ml_dtypes.bfloat16

_cache = {}

# head-interleave permutation: new feature f = 4c+h holds old feature 32h+c
_PERM = np.array([32 * h + c for c in range(C1) for h in range(HEADS)])


# ----------------------------------------------------------------------------
# Host-side graph preparation (indexing only)
# ----------------------------------------------------------------------------

def _prep(edge_index):
    src0 = np.asarray(edge_index[0], dtype=np.int64)
    dst0 = np.asarray(edge_index[1], dtype=np.int64)
    loop = np.arange(N_NODES, dtype=np.int64)
    src = np.concatenate([src0, loop]).astype(np.int64)
    dst = np.concatenate([dst0, loop]).astype(np.int64)

    deg = np.bincount(dst, minlength=N_NODES)  # includes self-loops
    order = np.argsort(-deg, kind="stable")  # nodes by in-degree desc

    # snake-deal global ranks to cores; contiguous blocks within core
    i = np.arange(N_NODES)
    r, j = i // NC, i % NC
    core_of_rank = np.where(r % 2 == 0, j, NC - 1 - j)
    block_of_rank = r // 128
    slot_of_rank = r % 128

    new_id = np.empty(N_NODES, dtype=np.int64)
    new_id[order] = (core_of_rank * NODES_PER_CORE + block_of_rank * 128
                     + slot_of_rank)
    old_of_new = np.full(N_PAD, -1, dtype=np.int64)
    old_of_new[new_id] = np.arange(N_NODES)

    # per-block round count: max degree in block b across all cores =
    # degree of the globally top-ranked node of the block
    R = np.array([int(deg[order[b * 128 * NC]]) for b in range(N_BLOCKS)])

    # schedule: snake-deal blocks (R desc) over NGROUPS groups
    groups = [[] for _ in range(NGROUPS)]
    for t, b in enumerate(range(N_BLOCKS)):
        row, col = t // NGROUPS, t % NGROUPS
        g = col if row % 2 == 0 else NGROUPS - 1 - col
        groups[g].append(b)
    groups = [tuple(g) for g in groups]
    sched = [b for g in groups for b in g]          # schedule order of blocks
    pos_of_block = np.empty(N_BLOCKS, dtype=np.int64)
    for p, b in enumerate(sched):
        pos_of_block[b] = p
    roff = np.zeros(N_BLOCKS, dtype=np.int64)       # round offset (sched order)
    off = 0
    for b in sched:
        roff[b] = off
        off += R[b]
    TOT_R = off

    # per-edge placement: round index k within each dst
    d_new = new_id[dst]
    s_new = new_id[src]
    o = np.argsort(d_new, kind="stable")
    d_s = d_new[o]
    seg_start = np.searchsorted(d_s, d_s, side="left")
    k_sorted = np.arange(len(d_s)) - seg_start
    k = np.empty(len(d_s), dtype=np.int64)
    k[o] = k_sorted

    core_e = d_new // NODES_PER_CORE
    dloc = d_new % NODES_PER_CORE
    blk = dloc // 128
    slot = dloc % 128
    col = roff[blk] + k

    srcg = np.full((NC, 128, TOT_R), N_PAD, dtype=np.int64)
    dstg = np.full((NC, 128, TOT_R), N_PAD, dtype=np.int64)
    srcg[core_e, slot, col] = s_new
    dstg[core_e, slot, col] = d_new

    return dict(TOT_R=TOT_R, R=tuple(int(x) for x in R), groups=tuple(groups),
                roff=roff, sched=sched, pos_of_block=pos_of_block,
                old_of_new=old_of_new, new_id=new_id, srcg=srcg, dstg=dstg)


def _w1ext(W1, att_src1, att_dst1):
    # [128, 136] : W1 (cols permuted to 4c+h) | asrc blockdiag (4) | adst (4)
    W1 = np.asarray(W1, np.float32)
    a_s = np.asarray(att_src1, np.float32)
    a_d = np.asarray(att_dst1, np.float32)
    asrc_bd = np.zeros((HID, HEADS), np.float32)
    adst_bd = np.zeros((HID, HEADS), np.float32)
    for h in range(HEADS):
        asrc_bd[32 * h:32 * h + 32, h] = a_s[h]
        adst_bd[32 * h:32 * h + 32, h] = a_d[h]
    ws = W1 @ asrc_bd
    wd = W1 @ adst_bd
    return np.concatenate([W1[:, _PERM], ws, wd], axis=1)


def _w2ext(W2, att_src2, att_dst2):
    # [128, 66] : W2 (rows permuted to 4c+h) | w2@a2s | w2@a2d
    W2 = np.asarray(W2, np.float32)
    a2s = np.asarray(att_src2, np.float32).reshape(-1)
    a2d = np.asarray(att_dst2, np.float32).reshape(-1)
    ws = (W2 @ a2s)[:, None]
    wd = (W2 @ a2d)[:, None]
    ext = np.concatenate([W2, ws, wd], axis=1)
    return ext[_PERM, :]


# ----------------------------------------------------------------------------
# numpy emulation of the device dataflow (for validation)
# ----------------------------------------------------------------------------

def _run_numpy(x, meta, W1e, W2e):
    TOT_R, R, roff = meta["TOT_R"], meta["R"], meta["roff"]
    pos_of_block = meta["pos_of_block"]
    xp = np.zeros((N_PAD, IN_DIM), np.float32)
    real = meta["old_of_new"] >= 0
    xp[real] = np.asarray(x, np.float32)[meta["old_of_new"][real]]
    xp = xp.astype(BF16).astype(np.float32)

    # Launch A
    feat = xp @ W1e.astype(BF16).astype(np.float32)  # [N_PAD, 136]
    hd_bf = feat[:, :128].astype(BF16)
    als, ald = feat[:, 128:132], feat[:, 132:136]
    hd_pad = np.concatenate([hd_bf, np.zeros((1, 128), BF16)], axis=0)
    als_pad = np.concatenate([als, np.zeros((1, 4), np.float32)], axis=0)
    ald_pad = np.concatenate([ald, np.full((1, 4), ZPAD, np.float32)], axis=0)

    h2a = np.zeros((N_PAD, 66), np.float32)
    out = np.zeros((N_PAD, OUT_DIM), np.float32)
    for c in range(NC):
        sg, dg = meta["srcg"][c], meta["dstg"][c]
        hdg = hd_pad[sg].astype(np.float32)            # [128, TOT_R, 128]
        z = (als_pad[sg] + ald_pad[dg]).astype(BF16).astype(np.float32)
        zm = np.maximum(z, NEG * z)
        ex = np.exp(zm).astype(BF16).astype(np.float32)  # [128, TOT_R, 4]
        hs = (hdg.reshape(128, TOT_R, 32, 4)
              * ex[:, :, None, :]).astype(BF16).astype(np.float32)
        hs = hs.reshape(128, TOT_R, 128)
        for b in range(N_BLOCKS):
            sl = slice(roff[b], roff[b] + R[b])
            agg = hs[:, sl].sum(axis=1)                 # [128, 128] f32
            den = ex[:, sl].sum(axis=1)                 # [128, 4] f32
            rd = (1.0 / den)
            hb = np.maximum(agg.reshape(128, 32, 4) * rd[:, None, :], 0.0)
            hb = hb.reshape(128, 128).astype(BF16)
            base = c * NODES_PER_CORE + b * 128
            h2a[base:base + 128] = (hb.astype(np.float32)
                                    @ W2e.astype(BF16).astype(np.float32))

    h2a_bf = h2a.astype(BF16).astype(np.float32)
    h2_bf = h2a_bf[:, :64].astype(BF16)
    als2, ald2 = h2a_bf[:, 64:65], h2a_bf[:, 65:66]
    h2_pad = np.concatenate([h2_bf, np.zeros((1, 64), BF16)], axis=0)
    als2_pad = np.concatenate([als2, np.zeros((1, 1), np.float32)], axis=0)
    ald2_pad = np.concatenate([ald2, np.full((1, 1), ZPAD, np.float32)], axis=0)

    for c in range(NC):
        sg, dg = meta["srcg"][c], meta["dstg"][c]
        h2g = h2_pad[sg].astype(np.float32)            # [128, TOT_R, 64]
        z = (als2_pad[sg] + ald2_pad[dg]).astype(BF16).astype(np.float32)
        zm = np.maximum(z, NEG * z)
        ex = np.exp(zm).astype(BF16).astype(np.float32)  # [128, TOT_R, 1]
        hs = (h2g * ex).astype(BF16).astype(np.float32)
        for b in range(N_BLOCKS):
            sl = slice(roff[b], roff[b] + R[b])
            agg = hs[:, sl].sum(axis=1)                 # [128, 64]
            den = ex[:, sl].sum(axis=1)                 # [128, 1]
            o = (agg / den).astype(BF16)
            base = c * NODES_PER_CORE + b * 128
            out[base:base + 128] = o

    res = np.zeros((N_NODES, OUT_DIM), np.float32)
    res[meta["old_of_new"][real]] = out[real]
    return res


# ----------------------------------------------------------------------------
# Bass programs
# ----------------------------------------------------------------------------

def _build_launch_a():
    import concourse.bacc as bacc
    import concourse.mybir as mybir
    import concourse.tile as tile

    nc = bacc.Bacc("TRN2", target_bir_lowering=False, debug=False, num_devices=NC)
    dt = mybir.dt
    xT = nc.dram_tensor("xT", [128, NODES_PER_CORE], dt.bfloat16, kind="ExternalInput")
    w1e = nc.dram_tensor("w1e", [128, 136], dt.bfloat16, kind="ExternalInput")
    hdT = nc.dram_tensor("hdT", [128, NODES_PER_CORE], dt.bfloat16, kind="ExternalOutput")
    aladT = nc.dram_tensor("aladT", [8, NODES_PER_CORE], dt.float32, kind="ExternalOutput")
    CS = 1792  # 7 chunks * 1792 = 12544; matmul tiles of 448 inside
    MS = 448
    with tile.TileContext(nc) as tc:
        with tc.tile_pool(name="w", bufs=1) as wp, \
             tc.tile_pool(name="x", bufs=3) as xp, \
             tc.tile_pool(name="o", bufs=3) as op_, \
             tc.tile_pool(name="ps", bufs=4, space="PSUM") as pp, \
             tc.tile_pool(name="ps2", bufs=2, space="PSUM") as pp2:
            wt = wp.tile([128, 136], dt.bfloat16)
            nc.sync.dma_start(wt[:], w1e.ap())
            for i in range(NODES_PER_CORE // CS):
                sl = slice(i * CS, (i + 1) * CS)
                xt = xp.tile([128, CS], dt.bfloat16, tag="x")
                (nc.sync if i % 2 == 0 else nc.scalar).dma_start(
                    xt[:], xT.ap()[:, sl])
                ot = op_.tile([128, CS], dt.bfloat16, tag="ot")
                ot2 = op_.tile([8, CS], dt.float32, tag="ot2")
                for j in range(CS // MS):
                    ms = slice(j * MS, (j + 1) * MS)
                    ps = pp.tile([128, MS], dt.float32, space="PSUM", tag="ps")
                    ps2 = pp2.tile([8, MS], dt.float32, space="PSUM", tag="ps2")
                    nc.tensor.matmul(ps[:], wt[:, 0:128], xt[:, ms],
                                     start=True, stop=True)
                    nc.tensor.matmul(ps2[:], wt[:, 128:136], xt[:, ms],
                                     start=True, stop=True)
                    # balance the big PSUM->SBUF casts across VEC and ACT
                    if j % 2 == 0:
                        nc.vector.tensor_copy(ot[:, ms], ps[:])
                        nc.scalar.copy(ot2[:, ms], ps2[:])
                    else:
                        nc.scalar.copy(ot[:, ms], ps[:])
                        nc.vector.tensor_copy(ot2[:, ms], ps2[:])
                (nc.scalar if i % 2 == 0 else nc.sync).dma_start(
                    hdT.ap()[:, sl], ot[:])
                nc.gpsimd.dma_start(aladT.ap()[:, sl], ot2[:])
    nc.compile()
    return nc


def _build_edge_launch(layer, R, groups, tot_r):
    """layer 1: F=128 4 heads + W2 epilogue; layer 2: F=64 1 head -> out2."""
    import concourse.bacc as bacc
    import concourse.mybir as mybir
    import concourse.tile as tile
    from concourse.masks import make_identity
    from contextlib import ExitStack

    F = 128 if layer == 1 else 64
    NH = HEADS if layer == 1 else 1
    EX8 = 8                      # ex replicated to 8 cols: long stride-1 runs
    FW = F + EX8
    C8 = F // EX8
    maxGR = max(sum(R[b] for b in g) for g in groups)
    maxB = max(len(g) for g in groups)
    ng = len(groups)

    nc = bacc.Bacc("TRN2", target_bir_lowering=False, debug=False, num_devices=NC)
    dt = mybir.dt
    hdg = nc.dram_tensor("hdg", [128, tot_r, F], dt.bfloat16, kind="ExternalInput")
    zg = nc.dram_tensor("zg", [128, tot_r, NH], dt.bfloat16, kind="ExternalInput")
    if layer == 1:
        w2e = nc.dram_tensor("w2e", [128, 66], dt.bfloat16, kind="ExternalInput")
        # schedule-order block columns; host unpermutes
        outt = nc.dram_tensor("h2a", [66, N_BLOCKS * 128], dt.bfloat16,
                              kind="ExternalOutput")
    else:
        outt = nc.dram_tensor("out2", [128, N_BLOCKS, OUT_DIM], dt.bfloat16,
                              kind="ExternalOutput")

    with tile.TileContext(nc) as tc, ExitStack() as ctx:
        cp = ctx.enter_context(tc.tile_pool(name="cst", bufs=1))
        hdp = ctx.enter_context(tc.tile_pool(name="hdp", bufs=3))
        zmp = ctx.enter_context(tc.tile_pool(name="zmp", bufs=2))
        hsp = ctx.enter_context(tc.tile_pool(name="hsp", bufs=2))
        ep = ctx.enter_context(tc.tile_pool(name="epi", bufs=2 * maxB))
        # one PSUM bank holds a whole group's aggs (maxB * FW * 4B <= 2KB)
        psa = ctx.enter_context(tc.tile_pool(name="psA", bufs=3, space="PSUM"))
        psb = ctx.enter_context(tc.tile_pool(name="psB", bufs=2, space="PSUM")) \
            if layer == 1 else None

        zga = cp.tile([128, tot_r, NH], dt.bfloat16)
        exa = cp.tile([128, tot_r, NH], dt.bfloat16)
        ident = cp.tile([128, 128], dt.bfloat16)
        make_identity(nc, ident[:])
        if layer == 1:
            w2t = cp.tile([128, 66], dt.bfloat16)
            nc.sync.dma_start(w2t[:], w2e.ap())
            stag = cp.tile([66, N_BLOCKS * 128], dt.bfloat16)
        else:
            stag = cp.tile([128, N_BLOCKS, OUT_DIM], dt.bfloat16)

        st = {}  # per-group live tiles for the software pipeline
        roff_l = {}
        off = 0
        for g in groups:
            for b in g:
                roff_l[b] = off
                off += R[b]
        pos_l = {}
        p = 0
        for g in groups:
            for b in g:
                pos_l[b] = p
                p += 1
        gstart = [roff_l[groups[g][0]] for g in range(ng)] + [tot_r]

        # geometric ex chunks: chunk ci (groups cbg[ci]..cbg[ci+1]) is issued
        # interleaved at pipeline stage ci, zga rides the idle gpsimd ring
        cbg = [0]
        w = 2
        while cbg[-1] + w < ng:
            cbg.append(cbg[-1] + w)
            w *= 2
        cbg.append(ng)
        NCHUNK = len(cbg) - 1
        maxCR = max(gstart[cbg[ci + 1]] - gstart[cbg[ci]]
                    for ci in range(NCHUNK))

        def prologue_chunk(ci):
            c0, c1 = gstart[cbg[ci]], gstart[cbg[ci + 1]]
            nc.gpsimd.dma_start(zga[:, c0:c1], zg.ap()[:, c0:c1])
            zmt = zmp.tile([128, maxCR, NH], dt.bfloat16, tag="zm")
            nc.vector.scalar_tensor_tensor(
                out=zmt[:, 0:c1 - c0], in0=zga[:, c0:c1], scalar=NEG,
                in1=zga[:, c0:c1],
                op0=mybir.AluOpType.mult, op1=mybir.AluOpType.max)
            nc.scalar.activation(exa[:, c0:c1], zmt[:, 0:c1 - c0],
                                 mybir.ActivationFunctionType.Exp)

        def load_compute(gi):
            blocks = groups[gi]
            r0 = roff_l[blocks[0]]
            GR = sum(R[b] for b in blocks)
            hdt = hdp.tile([128, maxGR, F], dt.bfloat16, tag="hd")
            (nc.sync if gi % 2 == 0 else nc.scalar).dma_start(
                hdt[:, 0:GR, :], hdg.ap()[:, r0:r0 + GR, :])
            hs = hsp.tile([128, maxGR, FW], dt.bfloat16, tag="hs")
            # replicate ex to 8 cols (den read from col F; feeds the multiply)
            nc.scalar.copy(
                hs[:, 0:GR, F:FW].rearrange("p r (k h) -> p r k h", h=NH),
                exa[:, r0:r0 + GR].unsqueeze(-2).to_broadcast(
                    [128, GR, EX8 // NH, NH]))
            exb = hs[:, 0:GR, F:FW].unsqueeze(-2).to_broadcast(
                [128, GR, C8, EX8])
            nc.vector.tensor_tensor(
                out=hs[:, 0:GR, 0:F].rearrange("p r (c e) -> p r c e", e=EX8),
                in0=hdt[:, 0:GR, :].rearrange("p r (c e) -> p r c e", e=EX8),
                in1=exb, op=mybir.AluOpType.mult)
            aggt = psa.tile([128, maxB, FW], dt.float32, space="PSUM", tag="agg")
            rr = 0
            for bi, b in enumerate(blocks):
                for k in range(R[b]):
                    nc.tensor.matmul(aggt[:, bi, :], ident[:], hs[:, rr + k, :],
                                     start=(k == 0), stop=(k == R[b] - 1))
                rr += R[b]
            st[gi] = {"aggt": aggt, "blocks": blocks}

        def epi1(gi):
            blocks = st[gi]["blocks"]
            nb = len(blocks)
            aggt = st[gi]["aggt"]
            rd = ep.tile([128, maxB, NH], dt.float32, tag="rd")
            nc.vector.reciprocal(rd[:, 0:nb, :], aggt[:, 0:nb, F:F + NH])
            if layer == 1:
                hrel = ep.tile([128, maxB, F], dt.bfloat16, tag="hrel")
                nc.scalar.activation(hrel[:, 0:nb, :], aggt[:, 0:nb, 0:F],
                                     mybir.ActivationFunctionType.Relu)
                hbf = ep.tile([128, maxB, F], dt.bfloat16, tag="hbf")
                rdx = rd[:, 0:nb, :].unsqueeze(-2).to_broadcast(
                    [128, nb, C1, NH])
                nc.vector.tensor_tensor(
                    out=hbf[:, 0:nb, :].rearrange("p b (c h) -> p b c h", h=NH),
                    in0=hrel[:, 0:nb, :].rearrange("p b (c h) -> p b c h", h=NH),
                    in1=rdx, op=mybir.AluOpType.mult)
                hTp = psb.tile([128, maxB, 128], dt.bfloat16, space="PSUM",
                               tag="hT")
                for bi in range(nb):
                    nc.tensor.transpose(hTp[:, bi, :], hbf[:, bi, :], ident[:])
                hTb = ep.tile([128, maxB, 128], dt.bfloat16, tag="hTb")
                nc.scalar.copy(hTb[:, 0:nb, :], hTp[:, 0:nb, :])
                st[gi]["hTb"] = hTb
            else:
                for bi, b in enumerate(blocks):
                    # per-dst scale is a per-partition scalar: ACT does it
                    nc.scalar.activation(stag[:, pos_l[b], :], aggt[:, bi, 0:F],
                                         mybir.ActivationFunctionType.Copy,
                                         scale=rd[:, bi, 0:1])
                p0, p1 = pos_l[blocks[0]], pos_l[blocks[-1]] + 1
                (nc.sync if gi % 2 == 0 else nc.scalar).dma_start(
                    outt.ap()[:, p0:p1, :], stag[:, p0:p1, :])
                del st[gi]

        def epi2(gi):
            blocks = st[gi]["blocks"]
            nb = len(blocks)
            hTb = st[gi]["hTb"]
            h2p = psb.tile([66, maxB * 128], dt.float32, space="PSUM", tag="h2a")
            nc.tensor.matmul(h2p[:, 0:nb * 128], w2t[:],
                             hTb[:, 0:nb, :].rearrange("p b f -> p (b f)"),
                             start=True, stop=True)
            p0, p1 = pos_l[blocks[0]] * 128, (pos_l[blocks[-1]] + 1) * 128
            nc.scalar.copy(stag[:, p0:p1], h2p[:, 0:nb * 128])
            (nc.sync if gi % 2 == 0 else nc.scalar).dma_start(
                outt.ap()[:, p0:p1], stag[:, p0:p1])
            del st[gi]

        for gi in range(ng + 3):
            if gi < NCHUNK:
                prologue_chunk(gi)
            if gi < ng:
                load_compute(gi)
            if 0 <= gi - 2 < ng:
                epi1(gi - 2)
            if layer == 1 and 0 <= gi - 3 < ng:
                epi2(gi - 3)
    nc.compile()
    return nc


# ----------------------------------------------------------------------------
# main entry
# ----------------------------------------------------------------------------

def kernel(x, edge_index, W1, att_src1, att_dst1, b1, W2, att_src2, att_dst2, b2):
    meta = _prep(edge_index)
    W1e = _w1ext(W1, att_src1, att_dst1)
    W2e = _w2ext(W2, att_src2, att_dst2)

    if os.environ.get("GAT_NUMPY"):
        return _run_numpy(x, meta, W1e, W2e)

    from concourse.bass_utils import run_bass_kernel_spmd

    TOT_R, R, groups = meta["TOT_R"], meta["R"], meta["groups"]
    old_of_new = meta["old_of_new"]
    real = old_of_new >= 0

    xp = np.zeros((N_PAD, IN_DIM), np.float32)
    xp[real] = np.asarray(x, np.float32)[old_of_new[real]]
    xp_bf = xp.astype(BF16)

    trace = bool(os.environ.get("GAT_TRACE"))
    times = []

    # ---- launch A
    nc_a = _get_cached("A2", _build_launch_a)
    in_maps = []
    for c in range(NC):
        sl = slice(c * NODES_PER_CORE, (c + 1) * NODES_PER_CORE)
        in_maps.append({"xT": np.ascontiguousarray(xp_bf[sl].T),
                        "w1e": W1e.astype(BF16)})
    res = run_bass_kernel_spmd(nc_a, in_maps, core_ids=list(range(NC)), trace=trace)
    times.append(res.exec_time_ns)
    hd_bf = np.concatenate([np.asarray(res.results[c]["hdT"]).T for c in range(NC)],
                           axis=0)
    alad = np.concatenate([res.results[c]["aladT"].T for c in range(NC)], axis=0)
    als, ald = alad[:, 0:4], alad[:, 4:8]

    hd_pad = np.concatenate([hd_bf, np.zeros((1, 128), BF16)], axis=0)
    als_pad = np.concatenate([als, np.zeros((1, 4), np.float32)], axis=0)
    ald_pad = np.concatenate([ald, np.full((1, 4), ZPAD, np.float32)], axis=0)

    # ---- launch B
    key_b = ("B2", TOT_R, R, groups)
    nc_b = _get_cached(key_b, lambda: _build_edge_launch(1, R, groups, TOT_R))
    in_maps = []
    for c in range(NC):
        sg, dg = meta["srcg"][c], meta["dstg"][c]
        z = (als_pad[sg] + ald_pad[dg]).astype(BF16)
        in_maps.append({
            "hdg": hd_pad[sg], "zg": z, "w2e": W2e.astype(BF16),
        })
    res = run_bass_kernel_spmd(nc_b, in_maps, core_ids=list(range(NC)), trace=trace)
    times.append(res.exec_time_ns)
    # h2a is in schedule order: column pos*128+s  ->  block sched[pos], slot s
    sched = meta["sched"]
    inv = np.empty(N_BLOCKS, dtype=np.int64)
    for p, b in enumerate(sched):
        inv[b] = p
    h2a_all = np.empty((N_PAD, 66), BF16)
    for c in range(NC):
        h2s = np.asarray(res.results[c]["h2a"]).T  # [98*128, 66] sched order
        h2s = h2s.reshape(N_BLOCKS, 128, 66)
        h2a_all[c * NODES_PER_CORE:(c + 1) * NODES_PER_CORE] = \
            h2s[inv].reshape(NODES_PER_CORE, 66)

    h2_bf = np.ascontiguousarray(h2a_all[:, :64])
    als2 = h2a_all[:, 64:65].astype(np.float32)
    ald2 = h2a_all[:, 65:66].astype(np.float32)
    h2_pad = np.concatenate([h2_bf, np.zeros((1, 64), BF16)], axis=0)
    als2_pad = np.concatenate([als2, np.zeros((1, 1), np.float32)], axis=0)
    ald2_pad = np.concatenate([ald2, np.full((1, 1), ZPAD, np.float32)], axis=0)

    # ---- launch C: merge group pairs (same flat block order -> same layout)
    groups_c = tuple(groups[i] + (groups[i + 1] if i + 1 < len(groups) else ())
                     for i in range(0, len(groups), 2))
    key_c = ("C2", TOT_R, R, groups_c)
    nc_c = _get_cached(key_c, lambda: _build_edge_launch(2, R, groups_c, TOT_R))
    in_maps = []
    for c in range(NC):
        sg, dg = meta["srcg"][c], meta["dstg"][c]
        z = (als2_pad[sg] + ald2_pad[dg]).astype(BF16)
        in_maps.append({
            "hdg": h2_pad[sg], "zg": z,
        })
    res = run_bass_kernel_spmd(nc_c, in_maps, core_ids=list(range(NC)), trace=trace)
    times.append(res.exec_time_ns)
    out_pad = np.empty((N_PAD, OUT_DIM), np.float32)
    for c in range(NC):
        o2 = np.asarray(res.results[c]["out2"]).astype(np.float32)  # [128, 98, 64]
        o2 = o2.transpose(1, 0, 2)  # [98 sched, 128 slot, 64]
        out_pad[c * NODES_PER_CORE:(c + 1) * NODES_PER_CORE] = \
            o2[inv].reshape(NODES_PER_CORE, OUT_DIM)

    if trace and all(t is not None for t in times):
        kernel.last_exec_ns = sum(times)
        print("per-launch exec ns:", times, "total:", sum(times))

    out = np.zeros((N_NODES, OUT_DIM), np.float32)
    out[old_of_new[real]] = out_pad[real]
    return out


def _get_cached(key, builder):
    if key not in _cache:
        _cache[key] = builder()
    return _cache[key]


# revision 38
# speedup vs baseline: 1.0619x; 1.0619x over previous
"""Trainium2 Bass kernel for a 2-layer GAT (PyG GATConv semantics).

Strategy (8 NeuronCores, SPMD, dst-sharded graph parallel, identity-rounds):
  - Host relabels nodes: global in-degree sort (incl. self-loop), snake-deal
    to 8 cores, contiguous 128-node blocks per core (98 blocks). Block b's
    round count R_b = max in-degree in block b across cores (identical
    program on all cores). Edge slot (block b, round k, dst slot j) holds
    the k-th in-edge of dst j; pad slots get hd=0 / z=-300.
  - With this layout the scatter matrix is the CONSTANT IDENTITY: the PE
    accumulates agg[j,:] += hs[round k][j,:] over rounds via PSUM
    accumulation (no per-edge one-hot stream, no per-tile stationary).
  - Launch A (dense): feat = W1ext.T @ x per core shard in bf16; W1ext
    packs W1 (columns permuted to f=4c+h interleaved head order) plus
    per-head attention columns. Outputs hd as bf16 and als/ald as f32.
  - Host gathers per-edge streams into [128 slot, TOT_R, F] grids:
    hd[src] bf16 and z = als[src]+ald[dst] bf16 (pad z = -300 -> ex ~ 0).
  - Launch B (L1 edge phase): per group of ~4 blocks: leaky = max(z, .2z),
    exp on ACT into hs ex columns, hs = hd * ex via one broadcast
    tensor_tensor (4c+h order keeps innermost stride 1 -> DVE 2x mode),
    identity matmuls accumulate agg|den in PSUM per block, epilogue:
    ACT reciprocal, fused relu+normalize, transpose, W2ext -> h2|als2|ald2.
  - Host gathers L2 per-edge streams; Launch C = L2 edge phase -> out2.
All FLOPs happen on device; the host only permutes/gathers/casts (and
adds the two gathered attention-logit streams).
"""

import os
import numpy as np
import ml_dtypes

N_NODES = 100000
N_EDGES = 1600000
IN_DIM = 128
HID = 128
HEADS = 4
C1 = 32
OUT_DIM = 64
NEG = 0.2
NC = 8
NODES_PER_CORE = 12544  # 98 blocks * 128
N_BLOCKS = 98
REAL_PER_CORE = 12500
N_PAD = NC * NODES_PER_CORE
NGROUPS = 33
ZPAD = -300.0

BF16 = ml_dtypes.bfloat16

_cache = {}

# head-interleave permutation: new feature f = 4c+h holds old feature 32h+c
_PERM = np.array([32 * h + c for c in range(C1) for h in range(HEADS)])


# ----------------------------------------------------------------------------
# Host-side graph preparation (indexing only)
# ----------------------------------------------------------------------------

def _prep(edge_index):
    src0 = np.asarray(edge_index[0], dtype=np.int64)
    dst0 = np.asarray(edge_index[1], dtype=np.int64)
    loop = np.arange(N_NODES, dtype=np.int64)
    src = np.concatenate([src0, loop]).astype(np.int64)
    dst = np.concatenate([dst0, loop]).astype(np.int64)

    deg = np.bincount(dst, minlength=N_NODES)  # includes self-loops
    order = np.argsort(-deg, kind="stable")  # nodes by in-degree desc

    # snake-deal global ranks to cores; contiguous blocks within core
    i = np.arange(N_NODES)
    r, j = i // NC, i % NC
    core_of_rank = np.where(r % 2 == 0, j, NC - 1 - j)
    block_of_rank = r // 128
    slot_of_rank = r % 128

    new_id = np.empty(N_NODES, dtype=np.int64)
    new_id[order] = (core_of_rank * NODES_PER_CORE + block_of_rank * 128
                     + slot_of_rank)
    old_of_new = np.full(N_PAD, -1, dtype=np.int64)
    old_of_new[new_id] = np.arange(N_NODES)

    # per-block round count: max degree in block b across all cores =
    # degree of the globally top-ranked node of the block
    R = np.array([int(deg[order[b * 128 * NC]]) for b in range(N_BLOCKS)])

    # schedule: snake-deal blocks (R desc) over NGROUPS groups
    groups = [[] for _ in range(NGROUPS)]
    for t, b in enumerate(range(N_BLOCKS)):
        row, col = t // NGROUPS, t % NGROUPS
        g = col if row % 2 == 0 else NGROUPS - 1 - col
        groups[g].append(b)
    groups = [tuple(g) for g in groups]
    sched = [b for g in groups for b in g]          # schedule order of blocks
    pos_of_block = np.empty(N_BLOCKS, dtype=np.int64)
    for p, b in enumerate(sched):
        pos_of_block[b] = p
    roff = np.zeros(N_BLOCKS, dtype=np.int64)       # round offset (sched order)
    off = 0
    for b in sched:
        roff[b] = off
        off += R[b]
    TOT_R = off

    # per-edge placement: round index k within each dst
    d_new = new_id[dst]
    s_new = new_id[src]
    o = np.argsort(d_new, kind="stable")
    d_s = d_new[o]
    seg_start = np.searchsorted(d_s, d_s, side="left")
    k_sorted = np.arange(len(d_s)) - seg_start
    k = np.empty(len(d_s), dtype=np.int64)
    k[o] = k_sorted

    core_e = d_new // NODES_PER_CORE
    dloc = d_new % NODES_PER_CORE
    blk = dloc // 128
    slot = dloc % 128
    col = roff[blk] + k

    srcg = np.full((NC, 128, TOT_R), N_PAD, dtype=np.int64)
    dstg = np.full((NC, 128, TOT_R), N_PAD, dtype=np.int64)
    srcg[core_e, slot, col] = s_new
    dstg[core_e, slot, col] = d_new

    return dict(TOT_R=TOT_R, R=tuple(int(x) for x in R), groups=tuple(groups),
                roff=roff, sched=sched, pos_of_block=pos_of_block,
                old_of_new=old_of_new, new_id=new_id, srcg=srcg, dstg=dstg)


def _w1ext(W1, att_src1, att_dst1):
    # [128, 136] : W1 (cols permuted to 4c+h) | asrc blockdiag (4) | adst (4)
    W1 = np.asarray(W1, np.float32)
    a_s = np.asarray(att_src1, np.float32)
    a_d = np.asarray(att_dst1, np.float32)
    asrc_bd = np.zeros((HID, HEADS), np.float32)
    adst_bd = np.zeros((HID, HEADS), np.float32)
    for h in range(HEADS):
        asrc_bd[32 * h:32 * h + 32, h] = a_s[h]
        adst_bd[32 * h:32 * h + 32, h] = a_d[h]
    ws = W1 @ asrc_bd
    wd = W1 @ adst_bd
    return np.concatenate([W1[:, _PERM], ws, wd], axis=1)


def _w2ext(W2, att_src2, att_dst2):
    # [128, 66] : W2 (rows permuted to 4c+h) | w2@a2s | w2@a2d
    W2 = np.asarray(W2, np.float32)
    a2s = np.asarray(att_src2, np.float32).reshape(-1)
    a2d = np.asarray(att_dst2, np.float32).reshape(-1)
    ws = (W2 @ a2s)[:, None]
    wd = (W2 @ a2d)[:, None]
    ext = np.concatenate([W2, ws, wd], axis=1)
    return ext[_PERM, :]


# ----------------------------------------------------------------------------
# numpy emulation of the device dataflow (for validation)
# ----------------------------------------------------------------------------

def _run_numpy(x, meta, W1e, W2e):
    TOT_R, R, roff = meta["TOT_R"], meta["R"], meta["roff"]
    pos_of_block = meta["pos_of_block"]
    xp = np.zeros((N_PAD, IN_DIM), np.float32)
    real = meta["old_of_new"] >= 0
    xp[real] = np.asarray(x, np.float32)[meta["old_of_new"][real]]
    xp = xp.astype(BF16).astype(np.float32)

    # Launch A
    feat = xp @ W1e.astype(BF16).astype(np.float32)  # [N_PAD, 136]
    hd_bf = feat[:, :128].astype(BF16)
    als, ald = feat[:, 128:132], feat[:, 132:136]
    hd_pad = np.concatenate([hd_bf, np.zeros((1, 128), BF16)], axis=0)
    als_pad = np.concatenate([als, np.zeros((1, 4), np.float32)], axis=0)
    ald_pad = np.concatenate([ald, np.full((1, 4), ZPAD, np.float32)], axis=0)

    h2a = np.zeros((N_PAD, 66), np.float32)
    out = np.zeros((N_PAD, OUT_DIM), np.float32)
    for c in range(NC):
        sg, dg = meta["srcg"][c], meta["dstg"][c]
        hdg = hd_pad[sg].astype(np.float32)            # [128, TOT_R, 128]
        z = (als_pad[sg] + ald_pad[dg]).astype(BF16).astype(np.float32)
        zm = np.maximum(z, NEG * z)
        ex = np.exp(zm).astype(BF16).astype(np.float32)  # [128, TOT_R, 4]
        hs = (hdg.reshape(128, TOT_R, 32, 4)
              * ex[:, :, None, :]).astype(BF16).astype(np.float32)
        hs = hs.reshape(128, TOT_R, 128)
        for b in range(N_BLOCKS):
            sl = slice(roff[b], roff[b] + R[b])
            agg = hs[:, sl].sum(axis=1)                 # [128, 128] f32
            den = ex[:, sl].sum(axis=1)                 # [128, 4] f32
            rd = (1.0 / den)
            hb = np.maximum(agg.reshape(128, 32, 4) * rd[:, None, :], 0.0)
            hb = hb.reshape(128, 128).astype(BF16)
            base = c * NODES_PER_CORE + b * 128
            h2a[base:base + 128] = (hb.astype(np.float32)
                                    @ W2e.astype(BF16).astype(np.float32))

    h2a_bf = h2a.astype(BF16).astype(np.float32)
    h2_bf = h2a_bf[:, :64].astype(BF16)
    als2, ald2 = h2a_bf[:, 64:65], h2a_bf[:, 65:66]
    h2_pad = np.concatenate([h2_bf, np.zeros((1, 64), BF16)], axis=0)
    als2_pad = np.concatenate([als2, np.zeros((1, 1), np.float32)], axis=0)
    ald2_pad = np.concatenate([ald2, np.full((1, 1), ZPAD, np.float32)], axis=0)

    for c in range(NC):
        sg, dg = meta["srcg"][c], meta["dstg"][c]
        h2g = h2_pad[sg].astype(np.float32)            # [128, TOT_R, 64]
        z = (als2_pad[sg] + ald2_pad[dg]).astype(BF16).astype(np.float32)
        zm = np.maximum(z, NEG * z)
        ex = np.exp(zm).astype(BF16).astype(np.float32)  # [128, TOT_R, 1]
        hs = (h2g * ex).astype(BF16).astype(np.float32)
        for b in range(N_BLOCKS):
            sl = slice(roff[b], roff[b] + R[b])
            agg = hs[:, sl].sum(axis=1)                 # [128, 64]
            den = ex[:, sl].sum(axis=1)                 # [128, 1]
            o = (agg / den).astype(BF16)
            base = c * NODES_PER_CORE + b * 128
            out[base:base + 128] = o

    res = np.zeros((N_NODES, OUT_DIM), np.float32)
    res[meta["old_of_new"][real]] = out[real]
    return res


# ----------------------------------------------------------------------------
# Bass programs
# ----------------------------------------------------------------------------

def _build_launch_a():
    import concourse.bacc as bacc
    import concourse.mybir as mybir
    import concourse.tile as tile

    nc = bacc.Bacc("TRN2", target_bir_lowering=False, debug=False, num_devices=NC)
    dt = mybir.dt
    xT = nc.dram_tensor("xT", [128, NODES_PER_CORE], dt.bfloat16, kind="ExternalInput")
    w1e = nc.dram_tensor("w1e", [128, 136], dt.bfloat16, kind="ExternalInput")
    hdT = nc.dram_tensor("hdT", [128, NODES_PER_CORE], dt.bfloat16, kind="ExternalOutput")
    aladT = nc.dram_tensor("aladT", [8, NODES_PER_CORE], dt.float32, kind="ExternalOutput")
    CS = 1792  # 7 chunks * 1792 = 12544; matmul tiles of 448 inside
    MS = 448
    with tile.TileContext(nc) as tc:
        with tc.tile_pool(name="w", bufs=1) as wp, \
             tc.tile_pool(name="x", bufs=3) as xp, \
             tc.tile_pool(name="o", bufs=3) as op_, \
             tc.tile_pool(name="ps", bufs=4, space="PSUM") as pp, \
             tc.tile_pool(name="ps2", bufs=2, space="PSUM") as pp2:
            wt = wp.tile([128, 136], dt.bfloat16)
            nc.sync.dma_start(wt[:], w1e.ap())
            for i in range(NODES_PER_CORE // CS):
                sl = slice(i * CS, (i + 1) * CS)
                xt = xp.tile([128, CS], dt.bfloat16, tag="x")
                (nc.sync if i % 2 == 0 else nc.scalar).dma_start(
                    xt[:], xT.ap()[:, sl])
                ot = op_.tile([128, CS], dt.bfloat16, tag="ot")
                ot2 = op_.tile([8, CS], dt.float32, tag="ot2")
                for j in range(CS // MS):
                    ms = slice(j * MS, (j + 1) * MS)
                    ps = pp.tile([128, MS], dt.float32, space="PSUM", tag="ps")
                    ps2 = pp2.tile([8, MS], dt.float32, space="PSUM", tag="ps2")
                    nc.tensor.matmul(ps[:], wt[:, 0:128], xt[:, ms],
                                     start=True, stop=True)
                    nc.tensor.matmul(ps2[:], wt[:, 128:136], xt[:, ms],
                                     start=True, stop=True)
                    # balance the big PSUM->SBUF casts across VEC and ACT
                    if j % 2 == 0:
                        nc.vector.tensor_copy(ot[:, ms], ps[:])
                        nc.scalar.copy(ot2[:, ms], ps2[:])
                    else:
                        nc.scalar.copy(ot[:, ms], ps[:])
                        nc.vector.tensor_copy(ot2[:, ms], ps2[:])
                (nc.scalar if i % 2 == 0 else nc.sync).dma_start(
                    hdT.ap()[:, sl], ot[:])
                nc.gpsimd.dma_start(aladT.ap()[:, sl], ot2[:])
    nc.compile()
    return nc


def _build_edge_launch(layer, R, groups, tot_r):
    """layer 1: F=128 4 heads + W2 epilogue; layer 2: F=64 1 head -> out2."""
    import concourse.bacc as bacc
    import concourse.mybir as mybir
    import concourse.tile as tile
    from concourse.masks import make_identity
    from contextlib import ExitStack

    F = 128 if layer == 1 else 64
    NH = HEADS if layer == 1 else 1
    EX8 = 8                      # ex replicated to 8 cols: long stride-1 runs
    FW = F + EX8
    C8 = F // EX8
    maxGR = max(sum(R[b] for b in g) for g in groups)
    maxB = max(len(g) for g in groups)
    ng = len(groups)

    nc = bacc.Bacc("TRN2", target_bir_lowering=False, debug=False, num_devices=NC)
    dt = mybir.dt
    hdg = nc.dram_tensor("hdg", [128, tot_r, F], dt.bfloat16, kind="ExternalInput")
    zg = nc.dram_tensor("zg", [128, tot_r, NH], dt.bfloat16, kind="ExternalInput")
    if layer == 1:
        w2e = nc.dram_tensor("w2e", [128, 66], dt.bfloat16, kind="ExternalInput")
        # schedule-order block columns; host unpermutes
        outt = nc.dram_tensor("h2a", [66, N_BLOCKS * 128], dt.bfloat16,
                              kind="ExternalOutput")
    else:
        outt = nc.dram_tensor("out2", [128, N_BLOCKS, OUT_DIM], dt.bfloat16,
                              kind="ExternalOutput")

    with tile.TileContext(nc) as tc, ExitStack() as ctx:
        cp = ctx.enter_context(tc.tile_pool(name="cst", bufs=1))
        hdp = ctx.enter_context(tc.tile_pool(name="hdp", bufs=3))
        zmp = ctx.enter_context(tc.tile_pool(name="zmp", bufs=2))
        hsp = ctx.enter_context(tc.tile_pool(name="hsp", bufs=2))
        ep = ctx.enter_context(tc.tile_pool(name="epi", bufs=2 * maxB))
        # one PSUM bank holds a whole group's aggs (maxB * FW * 4B <= 2KB)
        psa = ctx.enter_context(tc.tile_pool(name="psA", bufs=3, space="PSUM"))
        psb = ctx.enter_context(tc.tile_pool(name="psB", bufs=2, space="PSUM")) \
            if layer == 1 else None

        zga = cp.tile([128, tot_r, NH], dt.bfloat16)
        exa = cp.tile([128, tot_r, NH], dt.bfloat16)
        ident = cp.tile([128, 128], dt.bfloat16)
        make_identity(nc, ident[:])
        if layer == 1:
            w2t = cp.tile([128, 66], dt.bfloat16)
            nc.sync.dma_start(w2t[:], w2e.ap())
            stag = cp.tile([66, N_BLOCKS * 128], dt.bfloat16)
        else:
            stag = cp.tile([128, N_BLOCKS, OUT_DIM], dt.bfloat16)

        st = {}  # per-group live tiles for the software pipeline
        roff_l = {}
        off = 0
        for g in groups:
            for b in g:
                roff_l[b] = off
                off += R[b]
        pos_l = {}
        p = 0
        for g in groups:
            for b in g:
                pos_l[b] = p
                p += 1
        gstart = [roff_l[groups[g][0]] for g in range(ng)] + [tot_r]

        # ex prep in 2 chunks on the scalar load ring: a small head chunk so
        # group 0's multiply starts early, then the bulk right after the
        # first two hdg loads are in flight (never on gpsimd SWDGE: its
        # slow start + mid-stream SDMA stalls cost more than it saves)
        cbg = [0, min(4, ng), ng]
        NCHUNK = 2
        maxCR = gstart[ng] - gstart[cbg[1]]

        def prologue_chunk(ci):
            c0, c1 = gstart[cbg[ci]], gstart[cbg[ci + 1]]
            if c0 == c1:
                return
            nc.scalar.dma_start(zga[:, c0:c1], zg.ap()[:, c0:c1])
            zmt = zmp.tile([128, max(maxCR, gstart[cbg[1]]), NH],
                           dt.bfloat16, tag="zm")
            nc.vector.scalar_tensor_tensor(
                out=zmt[:, 0:c1 - c0], in0=zga[:, c0:c1], scalar=NEG,
                in1=zga[:, c0:c1],
                op0=mybir.AluOpType.mult, op1=mybir.AluOpType.max)
            nc.scalar.activation(exa[:, c0:c1], zmt[:, 0:c1 - c0],
                                 mybir.ActivationFunctionType.Exp)

        def load_compute(gi):
            blocks = groups[gi]
            r0 = roff_l[blocks[0]]
            GR = sum(R[b] for b in blocks)
            hdt = hdp.tile([128, maxGR, F], dt.bfloat16, tag="hd")
            (nc.sync if gi % 2 == 0 else nc.scalar).dma_start(
                hdt[:, 0:GR, :], hdg.ap()[:, r0:r0 + GR, :])
            hs = hsp.tile([128, maxGR, FW], dt.bfloat16, tag="hs")
            # replicate ex to 8 cols (den read from col F; feeds the multiply)
            nc.scalar.copy(
                hs[:, 0:GR, F:FW].rearrange("p r (k h) -> p r k h", h=NH),
                exa[:, r0:r0 + GR].unsqueeze(-2).to_broadcast(
                    [128, GR, EX8 // NH, NH]))
            exb = hs[:, 0:GR, F:FW].unsqueeze(-2).to_broadcast(
                [128, GR, C8, EX8])
            nc.vector.tensor_tensor(
                out=hs[:, 0:GR, 0:F].rearrange("p r (c e) -> p r c e", e=EX8),
                in0=hdt[:, 0:GR, :].rearrange("p r (c e) -> p r c e", e=EX8),
                in1=exb, op=mybir.AluOpType.mult)
            aggt = psa.tile([128, maxB, FW], dt.float32, space="PSUM", tag="agg")
            rr = 0
            for bi, b in enumerate(blocks):
                for k in range(R[b]):
                    nc.tensor.matmul(aggt[:, bi, :], ident[:], hs[:, rr + k, :],
                                     start=(k == 0), stop=(k == R[b] - 1))
                rr += R[b]
            st[gi] = {"aggt": aggt, "blocks": blocks}

        def epi1(gi):
            blocks = st[gi]["blocks"]
            nb = len(blocks)
            aggt = st[gi]["aggt"]
            rd = ep.tile([128, maxB, NH], dt.float32, tag="rd")
            nc.vector.reciprocal(rd[:, 0:nb, :], aggt[:, 0:nb, F:F + NH])
            if layer == 1:
                hrel = ep.tile([128, maxB, F], dt.bfloat16, tag="hrel")
                nc.scalar.activation(hrel[:, 0:nb, :], aggt[:, 0:nb, 0:F],
                                     mybir.ActivationFunctionType.Relu)
                hbf = ep.tile([128, maxB, F], dt.bfloat16, tag="hbf")
                rdx = rd[:, 0:nb, :].unsqueeze(-2).to_broadcast(
                    [128, nb, C1, NH])
                nc.vector.tensor_tensor(
                    out=hbf[:, 0:nb, :].rearrange("p b (c h) -> p b c h", h=NH),
                    in0=hrel[:, 0:nb, :].rearrange("p b (c h) -> p b c h", h=NH),
                    in1=rdx, op=mybir.AluOpType.mult)
                hTp = psb.tile([128, maxB, 128], dt.bfloat16, space="PSUM",
                               tag="hT")
                for bi in range(nb):
                    nc.tensor.transpose(hTp[:, bi, :], hbf[:, bi, :], ident[:])
                hTb = ep.tile([128, maxB, 128], dt.bfloat16, tag="hTb")
                nc.scalar.copy(hTb[:, 0:nb, :], hTp[:, 0:nb, :])
                st[gi]["hTb"] = hTb
            else:
                for bi, b in enumerate(blocks):
                    # per-dst scale is a per-partition scalar: ACT does it
                    nc.scalar.activation(stag[:, pos_l[b], :], aggt[:, bi, 0:F],
                                         mybir.ActivationFunctionType.Copy,
                                         scale=rd[:, bi, 0:1])
                p0, p1 = pos_l[blocks[0]], pos_l[blocks[-1]] + 1
                (nc.sync if gi % 2 == 0 else nc.scalar).dma_start(
                    outt.ap()[:, p0:p1, :], stag[:, p0:p1, :])
                del st[gi]

        def epi2(gi):
            blocks = st[gi]["blocks"]
            nb = len(blocks)
            hTb = st[gi]["hTb"]
            h2p = psb.tile([66, maxB * 128], dt.float32, space="PSUM", tag="h2a")
            nc.tensor.matmul(h2p[:, 0:nb * 128], w2t[:],
                             hTb[:, 0:nb, :].rearrange("p b f -> p (b f)"),
                             start=True, stop=True)
            p0, p1 = pos_l[blocks[0]] * 128, (pos_l[blocks[-1]] + 1) * 128
            nc.scalar.copy(stag[:, p0:p1], h2p[:, 0:nb * 128])
            (nc.sync if gi % 2 == 0 else nc.scalar).dma_start(
                outt.ap()[:, p0:p1], stag[:, p0:p1])
            del st[gi]

        prologue_chunk(0)
        for gi in range(ng + 3):
            if gi < ng:
                load_compute(gi)
            if gi == 1:
                prologue_chunk(1)
            if 0 <= gi - 2 < ng:
                epi1(gi - 2)
            if layer == 1 and 0 <= gi - 3 < ng:
                epi2(gi - 3)
    nc.compile()
    return nc


# ----------------------------------------------------------------------------
# main entry
# ----------------------------------------------------------------------------

def kernel(x, edge_index, W1, att_src1, att_dst1, b1, W2, att_src2, att_dst2, b2):
    meta = _prep(edge_index)
    W1e = _w1ext(W1, att_src1, att_dst1)
    W2e = _w2ext(W2, att_src2, att_dst2)

    if os.environ.get("GAT_NUMPY"):
        return _run_numpy(x, meta, W1e, W2e)

    from concourse.bass_utils import run_bass_kernel_spmd

    TOT_R, R, groups = meta["TOT_R"], meta["R"], meta["groups"]
    old_of_new = meta["old_of_new"]
    real = old_of_new >= 0

    xp = np.zeros((N_PAD, IN_DIM), np.float32)
    xp[real] = np.asarray(x, np.float32)[old_of_new[real]]
    xp_bf = xp.astype(BF16)

    trace = bool(os.environ.get("GAT_TRACE"))
    times = []

    # ---- launch A
    nc_a = _get_cached("A2", _build_launch_a)
    in_maps = []
    for c in range(NC):
        sl = slice(c * NODES_PER_CORE, (c + 1) * NODES_PER_CORE)
        in_maps.append({"xT": np.ascontiguousarray(xp_bf[sl].T),
                        "w1e": W1e.astype(BF16)})
    res = run_bass_kernel_spmd(nc_a, in_maps, core_ids=list(range(NC)), trace=trace)
    times.append(res.exec_time_ns)
    hd_bf = np.concatenate([np.asarray(res.results[c]["hdT"]).T for c in range(NC)],
                           axis=0)
    alad = np.concatenate([res.results[c]["aladT"].T for c in range(NC)], axis=0)
    als, ald = alad[:, 0:4], alad[:, 4:8]

    hd_pad = np.concatenate([hd_bf, np.zeros((1, 128), BF16)], axis=0)
    als_pad = np.concatenate([als, np.zeros((1, 4), np.float32)], axis=0)
    ald_pad = np.concatenate([ald, np.full((1, 4), ZPAD, np.float32)], axis=0)

    # ---- launch B
    key_b = ("B2", TOT_R, R, groups)
    nc_b = _get_cached(key_b, lambda: _build_edge_launch(1, R, groups, TOT_R))
    in_maps = []
    for c in range(NC):
        sg, dg = meta["srcg"][c], meta["dstg"][c]
        z = (als_pad[sg] + ald_pad[dg]).astype(BF16)
        in_maps.append({
            "hdg": hd_pad[sg], "zg": z, "w2e": W2e.astype(BF16),
        })
    res = run_bass_kernel_spmd(nc_b, in_maps, core_ids=list(range(NC)), trace=trace)
    times.append(res.exec_time_ns)
    # h2a is in schedule order: column pos*128+s  ->  block sched[pos], slot s
    sched = meta["sched"]
    inv = np.empty(N_BLOCKS, dtype=np.int64)
    for p, b in enumerate(sched):
        inv[b] = p
    h2a_all = np.empty((N_PAD, 66), BF16)
    for c in range(NC):
        h2s = np.asarray(res.results[c]["h2a"]).T  # [98*128, 66] sched order
        h2s = h2s.reshape(N_BLOCKS, 128, 66)
        h2a_all[c * NODES_PER_CORE:(c + 1) * NODES_PER_CORE] = \
            h2s[inv].reshape(NODES_PER_CORE, 66)

    h2_bf = np.ascontiguousarray(h2a_all[:, :64])
    als2 = h2a_all[:, 64:65].astype(np.float32)
    ald2 = h2a_all[:, 65:66].astype(np.float32)
    h2_pad = np.concatenate([h2_bf, np.zeros((1, 64), BF16)], axis=0)
    als2_pad = np.concatenate([als2, np.zeros((1, 1), np.float32)], axis=0)
    ald2_pad = np.concatenate([ald2, np.full((1, 1), ZPAD, np.float32)], axis=0)

    # ---- launch C: merge group pairs (same flat block order -> same layout)
    groups_c = tuple(groups[i] + (groups[i + 1] if i + 1 < len(groups) else ())
                     for i in range(0, len(groups), 2))
    key_c = ("C2", TOT_R, R, groups_c)
    nc_c = _get_cached(key_c, lambda: _build_edge_launch(2, R, groups_c, TOT_R))
    in_maps = []
    for c in range(NC):
        sg, dg = meta["srcg"][c], meta["dstg"][c]
        z = (als2_pad[sg] + ald2_pad[dg]).astype(BF16)
        in_maps.append({
            "hdg": h2_pad[sg], "zg": z,
        })
    res = run_bass_kernel_spmd(nc_c, in_maps, core_ids=list(range(NC)), trace=trace)
    times.append(res.exec_time_ns)
    out_pad = np.empty((N_PAD, OUT_DIM), np.float32)
    for c in range(NC):
        o2 = np.asarray(res.results[c]["out2"]).astype(np.float32)  # [128, 98, 64]
        o2 = o2.transpose(1, 0, 2)  # [98 sched, 128 slot, 64]
        out_pad[c * NODES_PER_CORE:(c + 1) * NODES_PER_CORE] = \
            o2[inv].reshape(NODES_PER_CORE, OUT_DIM)

    if trace and all(t is not None for t in times):
        kernel.last_exec_ns = sum(times)
        print("per-launch exec ns:", times, "total:", sum(times))

    out = np.zeros((N_NODES, OUT_DIM), np.float32)
    out[old_of_new[real]] = out_pad[real]
    return out


def _get_cached(key, builder):
    if key not in _cache:
        _cache[key] = builder()
    return _cache[key]
